# revision 4
# baseline (speedup 1.0000x reference)
"""Trainium2 Bass kernel for attention-weighted pooling.

Computes, for x[B,T,D], W[D,1], b[T,1]:
    et = tanh(x @ W + b)            # (B, T)
    at = softmax(et, axis=-1)       # (B, T)
    out = einsum('btd,bt->bd', x, at)

Sharding: pure data parallel over batch across 8 NeuronCores (4 batches per
core); W and b replicated. No collectives.

Key structure (per core, streaming single pass over x):
  - tanh output is bounded in [-1, 1], so softmax needs no max subtraction;
    exp() cannot overflow. Normalization by the denominator is deferred to
    the very end, so x is read from HBM exactly once (memory roofline).
  - x is cast fp32 -> fp16 during the DMA itself (SWDGE cast): HBM traffic
    stays the required 32 MiB/core of fp32, on-chip x is half the bytes,
    and the PE pooling matmul runs single-pass.
  - x chunks are DMA'd with partition-major layout "(p j) d": each of the
    128 partitions reads nj*2KB CONTIGUOUS bytes from HBM (vs 2KB strided
    lines), which gives large DMA packets and near-peak HBM read rate.
    b is pre-permuted to the matching t ordering (t = t0 + p*nj_tot + j);
    softmax/pooling are permutation-invariant over t so only b's layout
    must agree with x's.
  - Per 128x512 subtile, elin[t] = sum_d x[t,d]*W[d] is computed on THREE
    engines in parallel to stay under the DMA roofline:
      * DVE fused scalar_tensor_tensor (mult + accumulate, 1x uop)
      * DVE tensor_mul (2x fp16 uop) + ACT Copy-with-accumulator reduce
      * GPSIMD tensor_mul + one batched DVE tensor_reduce over the
        gpsimd products (innermost-axis reduce of [128, ng, 512])
  - ACT does tanh then exp (both in the same table set); PE accumulates
    p.T @ x_tile into PSUM [1, D] per batch.
  - Per-batch epilogue: S = sum_t p_t (ones-matmul), out = acc / S.
  - First chunk of batch 0 is small (512 KB) so compute starts ~2.5us in;
    last chunks of the last batch taper down so the post-DMA tail is short.
"""

import sys

sys.path.insert(0, "/opt/trn_rl_repo")

import numpy as np

B, T, D = 32, 4096, 512
N_CORES = 8
B_LOCAL = B // N_CORES          # 4 batches per core
P = 128                         # SBUF partitions
TS_T = 1024                     # t-rows per full super-tile (2 MiB fp32 DMA)
N_ST = T // TS_T                # 4 super-tiles per batch
N_J = TS_T // P                 # 8 t-subtiles per full super-tile

# per-chunk engine split: nj -> (n_dve_stt, n_dve_act, n_gpsimd)
SPLIT = {
    1: (1, 0, 0),
    2: (1, 1, 0),
    4: (1, 2, 1),
    6: (2, 2, 2),
    8: (2, 3, 3),
}

_PROGRAM = None


def _chunk_plans():
    full = [(t0, N_J) for t0 in range(0, T, TS_T)]
    first = [(0, 2), (256, 6)] + full[1:]
    last = full[:3] + [(3072, 4), (3584, 2), (3840, 1), (3968, 1)]
    plans = [first] + [list(full) for _ in range(B_LOCAL - 2)] + [last]
    return plans


def _build_program():
    import concourse.bacc as bacc
    import concourse.mybir as mybir
    import concourse.tile as tile

    f32 = mybir.dt.float32
    f16 = mybir.dt.float16
    nc = bacc.Bacc("TRN2", target_bir_lowering=False, debug=False)

    x_d = nc.dram_tensor("x", [B_LOCAL, T, D], f32, kind="ExternalInput")
    W_d = nc.dram_tensor("W", [D, 1], f32, kind="ExternalInput")
    b_d = nc.dram_tensor("b", [T, 1], f32, kind="ExternalInput")
    o_d = nc.dram_tensor("out", [B_LOCAL, D], f32, kind="ExternalOutput")

    plans = _chunk_plans()

    with tile.TileContext(nc) as tc:
        with (
            tc.tile_pool(name="consts", bufs=1) as consts,
            tc.tile_pool(name="xin", bufs=8) as xin,
            tc.tile_pool(name="scratch", bufs=3) as scratch_pool,
            tc.tile_pool(name="prod", bufs=4) as prod_pool,
            tc.tile_pool(name="gprod", bufs=3) as gprod_pool,
            tc.tile_pool(name="small", bufs=3) as small,
            tc.tile_pool(name="pbuf", bufs=2) as pbuf_pool,
            tc.tile_pool(name="acc_psum", bufs=2, space="PSUM") as acc_psum_pool,
            tc.tile_pool(name="s_psum", bufs=2, space="PSUM") as s_psum_pool,
        ):
            # W broadcast to all 128 partitions, cast to fp16: [128, D]
            w_bcast = consts.tile([P, D], f16)
            nc.gpsimd.dma_start(
                w_bcast[:],
                W_d.ap().rearrange("d one -> one d").broadcast_to([P, D]),
            )
            ones_col = consts.tile([P, 1], f32)
            nc.vector.memset(ones_col[:], 1.0)

            # b columns: for chunk (t0, nj), column col0+j (col0 = t0//P)
            # must hold b[t0 + p*nj + j] in partition p — i.e. partition p
            # reads nj contiguous floats at b[t0 + p*nj]. Chunk grids differ
            # between batches (taper), so load one b tile per distinct grid.
            b_tiles = {}
            for plan in plans:
                key = tuple(plan)
                if key in b_tiles:
                    continue
                bt = consts.tile(
                    [P, T // P, 1], f32, name=f"b_buf_{len(b_tiles)}"
                )
                for t0, nj in plan:
                    col0 = t0 // P
                    nc.sync.dma_start(
                        bt[:, col0 : col0 + nj, :],
                        b_d.ap()[t0 : t0 + nj * P, :].rearrange(
                            "(p j) one -> p j one", p=P
                        ),
                    )
                b_tiles[key] = bt

            for bb in range(B_LOCAL):
                p_buf = pbuf_pool.tile([P, T // P], f16)
                acc = acc_psum_pool.tile([1, D], f32)

                chunks = plans[bb]
                b_tile = b_tiles[tuple(chunks)]
                total_mm = sum(nj for _, nj in chunks)
                mm_idx = 0
                for t0, nj in chunks:
                    col0 = t0 // P
                    # SWDGE dma with inline fp32->fp16 cast; partition-major
                    # order: partition p holds rows t0+p*nj .. t0+p*nj+nj-1,
                    # i.e. nj*2KB contiguous HBM bytes per partition.
                    xt = xin.tile([P, nj, D], f16, tag="xt")
                    nc.gpsimd.dma_start(
                        xt[:],
                        x_d.ap()[bb, t0 : t0 + nj * P, :].rearrange(
                            "(p j) d -> p j d", p=P
                        ),
                    )
                    n_stt, n_act, n_gps = SPLIT[nj]
                    elin = small.tile([P, nj], f32)
                    for j in range(n_stt):
                        scratch = scratch_pool.tile([P, D], f16)
                        nc.vector.scalar_tensor_tensor(
                            out=scratch[:],
                            in0=xt[:, j, :],
                            scalar=1.0,
                            in1=w_bcast[:],
                            op0=mybir.AluOpType.mult,
                            op1=mybir.AluOpType.mult,
                            accum_out=elin[:, j : j + 1],
                        )
                    for j in range(n_stt, n_stt + n_act):
                        prod = prod_pool.tile([P, D], f16)
                        nc.vector.tensor_mul(prod[:], xt[:, j, :], w_bcast[:])
                        nc.scalar.activation(
                            prod[:],
                            prod[:],
                            mybir.ActivationFunctionType.Copy,
                            accum_out=elin[:, j : j + 1],
                        )
                    if n_gps:
                        g0 = n_stt + n_act
                        gprod = gprod_pool.tile([P, n_gps, D], f16, tag="gp")
                        for k in range(n_gps):
                            nc.gpsimd.tensor_tensor(
                                gprod[:, k, :],
                                xt[:, g0 + k, :],
                                w_bcast[:],
                                op=mybir.AluOpType.mult,
                            )
                        nc.vector.reduce_sum(
                            elin[:, g0 : g0 + n_gps],
                            gprod[:],
                            axis=mybir.AxisListType.X,
                        )
                    ee = small.tile([P, nj], f32)
                    nc.vector.tensor_add(
                        ee[:], elin[:], b_tile[:, col0 : col0 + nj, 0]
                    )
                    et = small.tile([P, nj], f32)
                    nc.scalar.activation(
                        et[:], ee[:], mybir.ActivationFunctionType.Tanh
                    )
                    nc.scalar.activation(
                        p_buf[:, col0 : col0 + nj],
                        et[:],
                        mybir.ActivationFunctionType.Exp,
                    )
                    for j in range(nj):
                        nc.tensor.matmul(
                            acc[:],
                            p_buf[:, col0 + j : col0 + j + 1],
                            xt[:, j, :],
                            start=(mm_idx == 0),
                            stop=(mm_idx == total_mm - 1),
                        )
                        mm_idx += 1

                # denominator S = sum_t p_t  (free-dim reduce, then
                # cross-partition reduce via ones-matmul)
                ssum = small.tile([P, 1], f32)
                nc.vector.reduce_sum(ssum[:], p_buf[:], axis=mybir.AxisListType.X)
                s_ps = s_psum_pool.tile([1, 1], f32)
                nc.tensor.matmul(s_ps[:], ssum[:], ones_col[:])
                sinv = small.tile([1, 1], f32)
                nc.vector.reciprocal(sinv[:], s_ps[:])
                out_sb = small.tile([1, D], f32)
                nc.scalar.mul(out_sb[:], acc[:], sinv[:])
                nc.sync.dma_start(o_d.ap()[bb : bb + 1, :], out_sb[:])

    nc.compile()
    return nc


def _get_program():
    global _PROGRAM
    if _PROGRAM is None:
        _PROGRAM = _build_program()
    return _PROGRAM


def _shard_inputs(x, W, b):
    x = np.ascontiguousarray(np.asarray(x, dtype=np.float32))
    W = np.ascontiguousarray(np.asarray(W, dtype=np.float32))
    b = np.ascontiguousarray(np.asarray(b, dtype=np.float32))
    return [
        {"x": x[c * B_LOCAL : (c + 1) * B_LOCAL], "W": W, "b": b}
        for c in range(N_CORES)
    ]


def _install_ntff_hook_shim():
    """The agent image's ``antenv`` lacks ``axon_hooks``, so the boot-time
    NTFF hook registration silently degrades. Recreate the module in
    sys.modules and register the ctypes hook against libaxon_pjrt.so."""
    import types

    if "antenv.axon_hooks" in sys.modules:
        return
    mod = types.ModuleType("antenv.axon_hooks")
    _hook = [None]
    mod.set_axon_ntff_profile_hook = lambda h: _hook.__setitem__(0, h)
    mod.get_axon_ntff_profile_hook = lambda: _hook[0]
    import antenv

    antenv.axon_hooks = mod
    sys.modules["antenv.axon_hooks"] = mod
    try:
        sys.path.insert(0, "/root/.axon_site")
        from trn_agent_boot.trn_boot import _ntff_profile_via_ctypes

        mod.set_axon_ntff_profile_hook(
            _ntff_profile_via_ctypes("/opt/axon/libaxon_pjrt.so")
        )
    except Exception as e:  # profiling is best-effort; run still works
        print(f"NTFF hook shim failed ({e}); tracing disabled", file=sys.stderr)


def _run(in_maps, trace=False):
    from concourse.bass_utils import run_bass_kernel_spmd

    nc = _get_program()
    kwargs = {}
    if trace:
        _install_ntff_hook_shim()
        kwargs = {"trace": True, "trace_cores": [0]}
    return run_bass_kernel_spmd(nc, in_maps, core_ids=list(range(N_CORES)), **kwargs)


def kernel(x, W, b):
    res = _run(_shard_inputs(x, W, b))
    return np.concatenate(
        [res.results[c]["out"] for c in range(N_CORES)], axis=0
    ).astype(np.float32)


def kernel_profiled(x, W, b):
    """Like kernel() but also returns the NTFF-measured exec time in ns."""
    res = _run(_shard_inputs(x, W, b), trace=True)
    out = np.concatenate(
        [res.results[c]["out"] for c in range(N_CORES)], axis=0
    ).astype(np.float32)
    return out, res


# revision 10
# speedup vs baseline: 1.1902x; 1.1902x over previous
"""Trainium2 Bass kernel for attention-weighted pooling.

Computes, for x[B,T,D], W[D,1], b[T,1]:
    et = tanh(x @ W + b)            # (B, T)
    at = softmax(et, axis=-1)       # (B, T)
    out = einsum('btd,bt->bd', x, at)

Sharding: pure data parallel over batch across 8 NeuronCores (4 batches per
core); W and b replicated. No collectives.

Key structure (per core, streaming single pass over x):
  - tanh output is bounded in [-1, 1], so softmax needs no max subtraction;
    exp() cannot overflow. Normalization by the denominator is deferred to
    the very end, so x is read from HBM exactly once (memory roofline).
  - x is cast fp32 -> fp16 during the DMA itself (SWDGE cast): HBM traffic
    stays the required 32 MiB/core of fp32, on-chip x is half the bytes,
    and the PE pooling matmul runs single-pass.
  - x chunks are DMA'd with partition-major layout "(p j) d": each of the
    128 partitions reads nj*2KB CONTIGUOUS bytes from HBM (vs 2KB strided
    lines), which gives large DMA packets and near-peak HBM read rate.
    b is pre-permuted to the matching t ordering (t = t0 + p*nj_tot + j);
    softmax/pooling are permutation-invariant over t so only b's layout
    must agree with x's.
  - Per chunk, elin[t] = sum_d x[t,d]*W[d] is computed with minimal
    engine-cycles (the part throttles on total activity, so energy ==
    time here):
      * ONE batched DVE tensor_mul [128, nj, 512] (2x fp16 uop) for all
        subtiles of the chunk (~290ns/subtile vs ~530 unbatched)
      * for k of the nj subtiles: in-place binary-halving tree of 2x
        tensor_adds (512->256->128->64) + one 1x tensor_reduce on the
        last 64 columns -> elin[:, 0:k]  (~450ns/subtile, all DVE)
      * for the rest: ACT Copy-with-accumulator reduce (~1.2us/subtile,
        keeps the scalar engine busy in parallel with the DVE tree)
  - ACT does tanh then exp (both in the same table set); PE accumulates
    p.T @ x_tile into PSUM [1, D] per batch.
  - Per-batch epilogue: S = sum_t p_t (ones-matmul), out = acc / S.
  - First chunk of batch 0 is small (512 KB) so compute starts ~2.5us in;
    last chunks of the last batch taper down so the post-DMA tail is short.
"""

import sys

sys.path.insert(0, "/opt/trn_rl_repo")

import numpy as np

B, T, D = 32, 4096, 512
N_CORES = 8
B_LOCAL = B // N_CORES          # 4 batches per core
P = 128                         # SBUF partitions
TS_T = 1024                     # t-rows per full super-tile (2 MiB fp32 DMA)
N_ST = T // TS_T                # 4 super-tiles per batch
N_J = TS_T // P                 # 8 t-subtiles per full super-tile

# per-chunk engine split: nj -> (n_dve_tree, n_act); nj == tree + act.
# Small tail chunks use fused STT / ACT paths instead (lowest latency).
SPLIT = {
    1: (0, 0),   # 1x STT
    2: (0, 1),   # 1x STT + 1x ACT
    4: (2, 2),
    6: (4, 2),
    8: (5, 3),
}

_PROGRAM = None


def _chunk_plans():
    full = [(t0, N_J) for t0 in range(0, T, TS_T)]
    first = [(0, 2), (256, 6)] + full[1:]
    last = full[:3] + [(3072, 4), (3584, 2), (3840, 1), (3968, 1)]
    plans = [first] + [list(full) for _ in range(B_LOCAL - 2)] + [last]
    return plans


def _build_program():
    import concourse.bacc as bacc
    import concourse.mybir as mybir
    import concourse.tile as tile

    f32 = mybir.dt.float32
    f16 = mybir.dt.float16
    nc = bacc.Bacc("TRN2", target_bir_lowering=False, debug=False)

    x_d = nc.dram_tensor("x", [B_LOCAL, T, D], f32, kind="ExternalInput")
    W_d = nc.dram_tensor("W", [D, 1], f32, kind="ExternalInput")
    b_d = nc.dram_tensor("b", [T, 1], f32, kind="ExternalInput")
    o_d = nc.dram_tensor("out", [B_LOCAL, D], f32, kind="ExternalOutput")

    plans = _chunk_plans()

    with tile.TileContext(nc) as tc:
        with (
            tc.tile_pool(name="consts", bufs=1) as consts,
            tc.tile_pool(name="xin", bufs=8) as xin,
            tc.tile_pool(name="scratch", bufs=3) as scratch_pool,
            tc.tile_pool(name="prod", bufs=3) as prod_pool,
            tc.tile_pool(name="small", bufs=3) as small,
            tc.tile_pool(name="pbuf", bufs=2) as pbuf_pool,
            tc.tile_pool(name="acc_psum", bufs=2, space="PSUM") as acc_psum_pool,
            tc.tile_pool(name="s_psum", bufs=2, space="PSUM") as s_psum_pool,
        ):
            # W broadcast to all 128 partitions, cast to fp16: [128, 1, D]
            # (middle singleton broadcasts over the nj axis of x chunks)
            w_bcast = consts.tile([P, 1, D], f16)
            nc.gpsimd.dma_start(
                w_bcast[:],
                W_d.ap().rearrange("(uno d) one -> one uno d", uno=1).broadcast_to(
                    [P, 1, D]
                ),
            )
            ones_col = consts.tile([P, 1], f32)
            nc.vector.memset(ones_col[:], 1.0)

            # b columns: for chunk (t0, nj), column col0+j (col0 = t0//P)
            # must hold b[t0 + p*nj + j] in partition p — i.e. partition p
            # reads nj contiguous floats at b[t0 + p*nj]. Chunk grids differ
            # between batches (taper), so load one b tile per distinct grid.
            b_tiles = {}
            for plan in plans:
                key = tuple(plan)
                if key in b_tiles:
                    continue
                bt = consts.tile(
                    [P, T // P, 1], f32, name=f"b_buf_{len(b_tiles)}"
                )
                for t0, nj in plan:
                    col0 = t0 // P
                    nc.sync.dma_start(
                        bt[:, col0 : col0 + nj, :],
                        b_d.ap()[t0 : t0 + nj * P, :].rearrange(
                            "(p j) one -> p j one", p=P
                        ),
                    )
                b_tiles[key] = bt

            for bb in range(B_LOCAL):
                p_buf = pbuf_pool.tile([P, T // P], f16)
                acc = acc_psum_pool.tile([1, D], f32)

                chunks = plans[bb]
                b_tile = b_tiles[tuple(chunks)]
                total_mm = sum(nj for _, nj in chunks)
                mm_idx = 0
                for t0, nj in chunks:
                    col0 = t0 // P
                    # SWDGE dma with inline fp32->fp16 cast; partition-major
                    # order: partition p holds rows t0+p*nj .. t0+p*nj+nj-1,
                    # i.e. nj*2KB contiguous HBM bytes per partition.
                    xt = xin.tile([P, nj, D], f16, tag="xt")
                    nc.gpsimd.dma_start(
                        xt[:],
                        x_d.ap()[bb, t0 : t0 + nj * P, :].rearrange(
                            "(p j) d -> p j d", p=P
                        ),
                    )
                    n_tree, n_act = SPLIT[nj]
                    elin = small.tile([P, nj], f32)
                    if nj <= 2:
                        # tail chunks: lowest-latency fused paths
                        scratch = scratch_pool.tile([P, D], f16)
                        nc.vector.scalar_tensor_tensor(
                            out=scratch[:],
                            in0=xt[:, 0, :],
                            scalar=1.0,
                            in1=w_bcast[:, 0, :],
                            op0=mybir.AluOpType.mult,
                            op1=mybir.AluOpType.mult,
                            accum_out=elin[:, 0:1],
                        )
                        for j in range(1, nj):
                            prod = prod_pool.tile([P, 1, D], f16, tag="prod")
                            nc.vector.tensor_mul(
                                prod[:, 0, :], xt[:, j, :], w_bcast[:, 0, :]
                            )
                            nc.scalar.activation(
                                prod[:, 0, :],
                                prod[:, 0, :],
                                mybir.ActivationFunctionType.Copy,
                                accum_out=elin[:, j : j + 1],
                            )
                    else:
                        # one batched 2x mult for the whole chunk
                        prod = prod_pool.tile([P, nj, D], f16, tag="prod")
                        nc.vector.tensor_mul(
                            prod[:], xt[:], w_bcast[:].broadcast_to([P, nj, D])
                        )
                        # DVE in-place halving tree on the first n_tree
                        # subtiles: 512->256->128->64 (2x adds), then one
                        # 1x reduce of the last 64 columns.
                        pk = prod[:, 0:n_tree, :]
                        for half in (256, 128, 64):
                            nc.vector.tensor_add(
                                pk[:, :, 0:half],
                                pk[:, :, 0:half],
                                pk[:, :, half : 2 * half],
                            )
                        nc.vector.reduce_sum(
                            elin[:, 0:n_tree],
                            pk[:, :, 0:64],
                            axis=mybir.AxisListType.X,
                        )
                        # ACT accumulator reduce for the rest
                        for j in range(n_tree, nj):
                            nc.scalar.activation(
                                prod[:, j, :],
                                prod[:, j, :],
                                mybir.ActivationFunctionType.Copy,
                                accum_out=elin[:, j : j + 1],
                            )
                    ee = small.tile([P, nj], f32)
                    nc.vector.tensor_add(
                        ee[:], elin[:], b_tile[:, col0 : col0 + nj, 0]
                    )
                    et = small.tile([P, nj], f32)
                    nc.scalar.activation(
                        et[:], ee[:], mybir.ActivationFunctionType.Tanh
                    )
                    nc.scalar.activation(
                        p_buf[:, col0 : col0 + nj],
                        et[:],
                        mybir.ActivationFunctionType.Exp,
                    )
                    for j in range(nj):
                        nc.tensor.matmul(
                            acc[:],
                            p_buf[:, col0 + j : col0 + j + 1],
                            xt[:, j, :],
                            start=(mm_idx == 0),
                            stop=(mm_idx == total_mm - 1),
                        )
                        mm_idx += 1

                # denominator S = sum_t p_t  (free-dim reduce, then
                # cross-partition reduce via ones-matmul)
                ssum = small.tile([P, 1], f32)
                nc.vector.reduce_sum(ssum[:], p_buf[:], axis=mybir.AxisListType.X)
                s_ps = s_psum_pool.tile([1, 1], f32)
                nc.tensor.matmul(s_ps[:], ssum[:], ones_col[:])
                sinv = small.tile([1, 1], f32)
                nc.vector.reciprocal(sinv[:], s_ps[:])
                out_sb = small.tile([1, D], f32)
                nc.scalar.mul(out_sb[:], acc[:], sinv[:])
                nc.sync.dma_start(o_d.ap()[bb : bb + 1, :], out_sb[:])

    nc.compile()
    return nc


def _get_program():
    global _PROGRAM
    if _PROGRAM is None:
        _PROGRAM = _build_program()
    return _PROGRAM


def _shard_inputs(x, W, b):
    x = np.ascontiguousarray(np.asarray(x, dtype=np.float32))
    W = np.ascontiguousarray(np.asarray(W, dtype=np.float32))
    b = np.ascontiguousarray(np.asarray(b, dtype=np.float32))
    return [
        {"x": x[c * B_LOCAL : (c + 1) * B_LOCAL], "W": W, "b": b}
        for c in range(N_CORES)
    ]


def _install_ntff_hook_shim():
    """The agent image's ``antenv`` lacks ``axon_hooks``, so the boot-time
    NTFF hook registration silently degrades. Recreate the module in
    sys.modules and register the ctypes hook against libaxon_pjrt.so."""
    import types

    if "antenv.axon_hooks" in sys.modules:
        return
    mod = types.ModuleType("antenv.axon_hooks")
    _hook = [None]
    mod.set_axon_ntff_profile_hook = lambda h: _hook.__setitem__(0, h)
    mod.get_axon_ntff_profile_hook = lambda: _hook[0]
    import antenv

    antenv.axon_hooks = mod
    sys.modules["antenv.axon_hooks"] = mod
    try:
        sys.path.insert(0, "/root/.axon_site")
        from trn_agent_boot.trn_boot import _ntff_profile_via_ctypes

        mod.set_axon_ntff_profile_hook(
            _ntff_profile_via_ctypes("/opt/axon/libaxon_pjrt.so")
        )
    except Exception as e:  # profiling is best-effort; run still works
        print(f"NTFF hook shim failed ({e}); tracing disabled", file=sys.stderr)


def _run(in_maps, trace=False):
    from concourse.bass_utils import run_bass_kernel_spmd

    nc = _get_program()
    kwargs = {}
    if trace:
        _install_ntff_hook_shim()
        kwargs = {"trace": True, "trace_cores": [0]}
    return run_bass_kernel_spmd(nc, in_maps, core_ids=list(range(N_CORES)), **kwargs)


def kernel(x, W, b):
    res = _run(_shard_inputs(x, W, b))
    return np.concatenate(
        [res.results[c]["out"] for c in range(N_CORES)], axis=0
    ).astype(np.float32)


def kernel_profiled(x, W, b):
    """Like kernel() but also returns the NTFF-measured exec time in ns."""
    res = _run(_shard_inputs(x, W, b), trace=True)
    out = np.concatenate(
        [res.results[c]["out"] for c in range(N_CORES)], axis=0
    ).astype(np.float32)
    return out, res
